# revision 40
# baseline (speedup 1.0000x reference)
"""Trainium2 Bass kernel for nn_MessagePassingLayer (gnn_message_passing).

Computes, for x:[B,C,N,1] f32, edge_index:[B,N,K] i32, alpha scalar:
    out[b,c,n] = x[b,c,n]*(1+alpha) + sum_k x[b,c,edge_index[b,n,k]]

Sharding: B=8 batch samples, one per NeuronCore (data parallel). Edge
indices are intra-sample so there is no cross-core communication.

Modes (KERNEL_MODE):
  hbm (default): node-major f32 HBM row gather, 4 SWDGE queues, with a
    tail-split issue schedule (KERNEL_TAILSPLIT=1 default): the last
    wave uses 1024-idx gathers so the post-generation drain tail halves.
  sbuf/hbmt: experimental bf16 transpose-gather modes (correct only on
    a single queue due to shared-xbar pairing corruption; slow).
"""
import os
import sys
import types

import numpy as np

B, C, N, K = 8, 64, 4096, 16
NCORES = 8
P = 128
COLS = N // P  # 32 nodes per partition
FREE = COLS * C  # 2048 f32 per partition

LAST_EXEC_NS = None

MODE = os.environ.get("KERNEL_MODE", "hbm")


# ---------------------------------------------------------------------------
# axon NTFF profile hook shim (the agent image's antenv lacks axon_hooks)
# ---------------------------------------------------------------------------
def _install_profile_shim():
    if "antenv.axon_hooks" in sys.modules:
        return
    try:
        import antenv

        mod = types.ModuleType("antenv.axon_hooks")
        mod._hook = None
        mod.set_axon_ntff_profile_hook = lambda h: setattr(mod, "_hook", h)
        mod.get_axon_ntff_profile_hook = lambda: mod._hook
        sys.modules["antenv.axon_hooks"] = mod
        antenv.axon_hooks = mod
        from trn_agent_boot.trn_boot import _ntff_profile_via_ctypes

        mod.set_axon_ntff_profile_hook(
            _ntff_profile_via_ctypes("/opt/axon/libaxon_pjrt.so")
        )
    except Exception:
        pass


# ---------------------------------------------------------------------------
# Walrus in this container rejects >1 sync-wait per instruction. Split any
# multi-wait instruction into single-wait NoOps on the same engine.
# ---------------------------------------------------------------------------
def _split_multiwaits(nc, mybir):
    cnt = [0]
    for f in nc.m.functions:
        for bb in f.blocks:
            new_list = []
            for ins in bb.instructions:
                si = ins.sync_info
                if si is not None and si.on_wait and len(si.on_wait) > 1:
                    waits = list(si.on_wait)
                    for w in waits[:-1]:
                        cnt[0] += 1
                        nop = mybir.InstNoOp(name=f"I-waitsplit-{cnt[0]}")
                        nop.engine = ins.engine
                        nop.sync_info = mybir.SyncInfo(on_wait=[w], on_update=[])
                        try:
                            nc.register_instruction(nop, overwrite=True)
                        except Exception:
                            pass
                        new_list.append(nop)
                    ins.sync_info = mybir.SyncInfo(
                        on_wait=[waits[-1]], on_update=list(si.on_update)
                    )
                new_list.append(ins)
            bb.instructions = new_list


# ---------------------------------------------------------------------------
# Device program — sbuf/hbmt transpose-gather modes
# ---------------------------------------------------------------------------
GATHER_CHUNK = int(os.environ.get("KERNEL_GATHER_CHUNK", "2048"))
SCRATCH = int(os.environ.get("KERNEL_SCRATCH", "32768"))
NQUEUES = int(os.environ.get("KERNEL_QUEUES", "4"))

CPK = N // GATHER_CHUNK          # chunks per k
IPC = GATHER_CHUNK // 16         # idx cols per chunk (16-partition wrap)
NBLK = K * CPK                   # total gather instructions


def _build_program_t(src_sbuf: bool):
    import concourse.mybir as mybir
    import concourse.tile as tile
    from concourse import bacc

    nc = bacc.Bacc("TRN2", target_bir_lowering=False, debug=False,
                   num_devices=NCORES, num_swdge_queues=NQUEUES,
                   dynamic_dma_scratch_size=SCRATCH)
    bf16 = mybir.dt.bfloat16
    f32 = mybir.dt.float32
    # token table: node t -> 256B = 128 bf16 (channels 0..63 duplicated).
    # SBUF layout: partition t%128, free cols [ (t//128)*128, +128 ).
    # HBM layout (hbmt): row-major [4096, 128].
    if src_sbuf:
        tab_d = nc.dram_tensor("tab", [P, N // P * P], bf16,
                               kind="ExternalInput")
    else:
        tab_d = nc.dram_tensor("tab", [N, P], bf16, kind="ExternalInput")
    idx_d = nc.dram_tensor("idx", [P, NBLK * IPC], mybir.dt.int16,
                           kind="ExternalInput")
    x_d = nc.dram_tensor("x", [C, N], f32, kind="ExternalInput")
    al_d = nc.dram_tensor("alpha", [P, 1], f32, kind="ExternalInput")
    out_d = nc.dram_tensor("out", [C, N], f32, kind="ExternalOutput")

    with tile.TileContext(nc) as tc:
        with tc.tile_pool(name="sbuf", bufs=1) as pool:
            idx_sb = pool.tile([P, NBLK * IPC], mybir.dt.int16, tag="idx")
            x_sb = pool.tile([C, N], f32, tag="x")
            al_sb = pool.tile([P, 1], f32, tag="al")
            g = [[pool.tile([P, GATHER_CHUNK], bf16, tag=f"g{k}_{c}",
                            name=f"g{k}_{c}") for c in range(CPK)]
                 for k in range(K)]
            if src_sbuf:
                tab_sb = pool.tile([P, N], bf16, tag="tab")
                nc.sync.dma_start(out=tab_sb[:], in_=tab_d.ap())

            # idx chunk-0 blocks first so the first gather wave starts ASAP
            nc.scalar.dma_start(out=idx_sb[:, :K * IPC],
                                in_=idx_d.ap()[:, :K * IPC])
            nreg = nc.gpsimd.to_reg(GATHER_CHUNK)

            gi = 0
            for c in range(CPK):
                for k in range(K):
                    blk = c * K + k
                    src = tab_sb[:] if src_sbuf else tab_d.ap()
                    kw = dict(sbuf_tokens_per_rank=P,
                              sbuf_free_dim_per_rank=256) if src_sbuf else {}
                    nc.gpsimd.dma_gather(
                        out_ap=g[k][c][:].rearrange("p (a n) -> p a n", a=1),
                        in_ap=src,
                        idxs_ap=idx_sb[:, blk * IPC:(blk + 1) * IPC],
                        num_idxs=GATHER_CHUNK,
                        num_idxs_reg=nreg,
                        elem_size=P,
                        transpose=True,
                        queue_num=gi % NQUEUES,
                        single_packet=False,
                        **kw,
                    )
                    gi += 1
                if c == 0:
                    # remaining loads, issued behind the first gather wave
                    nc.scalar.dma_start(out=al_sb[:], in_=al_d.ap())
                    if CPK > 1:
                        nc.scalar.dma_start(
                            out=idx_sb[:, K * IPC:],
                            in_=idx_d.ap()[:, K * IPC:])
                    nc.scalar.dma_start(out=x_sb[:], in_=x_d.ap())
                    nc.scalar.add(out=al_sb[:], in_=al_sb[:], add=1.0)
                    # x*(1+alpha) on DVE while gathers run
                    nc.vector.tensor_scalar_mul(
                        out=x_sb[:], in0=x_sb[:], scalar1=al_sb[:C, :1])

            # pairwise tree sum per chunk (bf16), then out = xs + m
            for c in range(CPK):
                for step in (1, 2, 4, 8):
                    for a in range(0, K, 2 * step):
                        nc.vector.tensor_add(
                            out=g[a][c][:], in0=g[a][c][:],
                            in1=g[a + step][c][:])
                lo, hi = c * GATHER_CHUNK, (c + 1) * GATHER_CHUNK
                nc.vector.tensor_add(
                    out=x_sb[:, lo:hi], in0=x_sb[:, lo:hi],
                    in1=g[0][c][:C, :])
                nc.sync.dma_start(out=out_d.ap()[:, lo:hi],
                                  in_=x_sb[:, lo:hi])

    nc.compile()
    _split_multiwaits(nc, mybir)
    return nc


# ---------------------------------------------------------------------------
# Post-compile pass: give every SWDGE gather a private completion semaphore.
#
# The tile framework round-robins all Pool DMA completions onto 8 DMASW lane
# sems; lane reuse puts a wait on each gather past the 8th, which head-blocks
# the GpSimd sequencer and caps DMA in-flight depth (~71% SDMA duty in the
# trace). With one sem per gather there is no reuse wait: issue depth is
# bounded only by the descriptor rings, and SDMA stays fed.
# ---------------------------------------------------------------------------
def _privatize_gather_sems(nc, mybir, spare_ids):
    import dataclasses

    count = {}   # lane sem id -> max generation seen
    remap = {}   # (lane, gen) -> private sem id
    nxt = 0
    for f in nc.m.functions:
        for bb in f.blocks:
            for ins in bb.instructions:
                if type(ins).__name__ not in ("InstDMAGatherAnt",
                                              "InstDMACopy"):
                    continue
                if str(getattr(ins, "engine", "")) != "EngineType.Pool":
                    continue
                si = ins.sync_info
                upd = [u for u in si.on_update
                       if u.sync_type == "semaphore"
                       and (u.ant_name or "").startswith("DMASW")]
                if not upd:
                    continue
                u = upd[0]
                lane = u.id
                # generation = (value of the reuse wait on its own lane)/16+1;
                # gen 1 has no reuse wait. Iteration order is NOT schedule
                # order, so the wait value is the only reliable source.
                reuse = [w for w in si.on_wait
                         if w.sync_type == "semaphore" and w.id == lane]
                gen = (reuse[0].wait_value // 16 + 1) if reuse else 1
                count[lane] = max(count.get(lane, 0), gen)
                if gen == 1:
                    continue
                new_id = spare_ids[nxt]
                nxt += 1
                remap[(lane, gen)] = new_id
                new_upds = [
                    dataclasses.replace(x, id=new_id, ant_name=f"GPRIV{nxt}")
                    if x is u else x
                    for x in si.on_update
                ]
                new_waits = [w for w in si.on_wait
                             if not (w.sync_type == "semaphore"
                                     and w.id == lane)]
                ins.sync_info = mybir.SyncInfo(on_wait=new_waits,
                                               on_update=new_upds)
    # consumers + teardown: (lane, 16k) -> (lane, 16) + privates of gens 2..k
    for f in nc.m.functions:
        for bb in f.blocks:
            for ins in bb.instructions:
                si = ins.sync_info
                if si is None or not si.on_wait:
                    continue
                if type(ins).__name__ == "InstDMAGatherAnt":
                    continue
                waits, changed = [], False
                for w in si.on_wait:
                    if (w.sync_type == "semaphore" and w.id in count
                            and w.wait_mode == "sem-ge-imm"
                            and w.wait_value % 16 == 0):
                        k = w.wait_value // 16
                        if k >= 2:
                            changed = True
                            waits.append(dataclasses.replace(w, wait_value=16))
                            for j in range(2, k + 1):
                                waits.append(dataclasses.replace(
                                    w, id=remap[(w.id, j)],
                                    ant_name=f"GPRIVW{w.id}_{j}",
                                    wait_value=16))
                            continue
                    waits.append(w)
                if changed:
                    ins.sync_info = mybir.SyncInfo(
                        on_wait=waits, on_update=list(si.on_update))
    return nxt


# ---------------------------------------------------------------------------
# Tail-split variant: identical to hbm mode, but the final issue wave uses
# 1024-idx gathers so the post-generation drain tail halves (~5 µs).
# Blocks are (k, lo_slot, n_slots) in issue order; lo must be 0 mod 128.
# ---------------------------------------------------------------------------
_TS_BLOCKS = (
    [(k, 0, 2048) for k in range(16)]
    + [(k, 2048, 2048) for k in range(12)]
    + [(k, 2048, 1024) for k in range(12, 16)]
    + [(k, 3072, 1024) for k in range(12, 16)]
)


def _prep_idx_blocks(edge_b):
    parts = []
    for (k, lo, ln) in _TS_BLOCKS:
        ids = edge_b[_PERM[lo:lo + ln], k].astype(np.int16)   # [ln]
        w = ids.reshape(ln // 16, 16).T                       # [16, ln/16]
        parts.append(np.tile(w, (8, 1)))                      # [128, ln/16]
    return np.ascontiguousarray(np.concatenate(parts, axis=1))


# Big-chunk variant: flat global slot space G = k*4096 + slot, cut into
# 3968-slot instructions (ring fits 3968/16+1=249 <= 256 descs/engine;
# only 4096 -> 257 overflows). 17 instructions instead of 32.
_BC_CHUNK = 3968
_BC_BOUNDS = list(range(0, N * K, _BC_CHUNK)) + [N * K]


def _bc_pieces(j):
    """Instruction j's (k, i_lo, i_len) pieces in the per-k slot space."""
    glo, ghi = _BC_BOUNDS[j], _BC_BOUNDS[j + 1]
    out = []
    g = glo
    while g < ghi:
        k, i = g // N, g % N
        ln = min(ghi - g, N - i)
        out.append((k, i, ln))
        g += ln
    return out


def _prep_idx_bc(edge_b):
    parts = []
    for j in range(len(_BC_BOUNDS) - 1):
        vals = []
        for (k, lo, ln) in _bc_pieces(j):
            vals.append(edge_b[_PERM[lo:lo + ln], k].astype(np.int16))
        v = np.concatenate(vals)                      # [inst slots]
        w = v.reshape(len(v) // 16, 16).T             # [16, slots/16]
        parts.append(np.tile(w, (8, 1)))
    return np.ascontiguousarray(np.concatenate(parts, axis=1))


def _build_program_hbm_bc():
    import concourse.mybir as mybir
    import concourse.tile as tile
    from concourse import bacc

    nc = bacc.Bacc("TRN2", target_bir_lowering=False, debug=False,
                   num_devices=NCORES, num_swdge_queues=4,
                   dynamic_dma_scratch_size=16384)
    nins = len(_BC_BOUNDS) - 1
    sizes = [_BC_BOUNDS[j + 1] - _BC_BOUNDS[j] for j in range(nins)]
    total_cols = sum(s // 16 for s in sizes)
    xt_d = nc.dram_tensor("xt", [N, C], mybir.dt.float32, kind="ExternalInput")
    idx_d = nc.dram_tensor("idx", [P, total_cols], mybir.dt.int16,
                           kind="ExternalInput")
    alpha_d = nc.dram_tensor("alpha", [P, 1], mybir.dt.float32,
                             kind="ExternalInput")
    out_d = nc.dram_tensor("out", [N, C], mybir.dt.float32,
                           kind="ExternalOutput")

    segw = (2048 // P) * C
    offs = []
    off = 0
    for s in sizes:
        offs.append(off)
        off += s // 16

    with tile.TileContext(nc) as tc:
        with tc.tile_pool(name="sbuf", bufs=1) as pool:
            xt_sb = pool.tile([P, FREE], mybir.dt.float32, tag="xt")
            idx_sb = pool.tile([P, total_cols], mybir.dt.int16, tag="idx")
            al_sb = pool.tile([P, 1], mybir.dt.float32, tag="al")
            g = [pool.tile([P, (s // P) * C], mybir.dt.float32,
                           tag=f"g{j}", name=f"g{j}")
                 for j, s in enumerate(sizes)]
            o = [pool.tile([P, segw], mybir.dt.float32, tag=f"o{c}",
                           name=f"o{c}") for c in range(2)]

            xt_nm = xt_d.ap().rearrange("(p a) c -> p (a c)", p=P)
            out_nm = out_d.ap().rearrange("(p a) c -> p (a c)", p=P)

            b4 = offs[4] if nins > 4 else total_cols
            nc.sync.dma_start(out=idx_sb[:, :b4], in_=idx_d.ap()[:, :b4])
            if b4 < total_cols:
                nc.scalar.dma_start(out=idx_sb[:, b4:],
                                    in_=idx_d.ap()[:, b4:])
            regs = {s: nc.gpsimd.to_reg(s) for s in sorted(set(sizes))}

            for j, s in enumerate(sizes):
                nc.gpsimd.dma_gather(
                    out_ap=g[j][:].rearrange("p (a c) -> p a c", c=C),
                    in_ap=xt_d.ap(),
                    idxs_ap=idx_sb[:, offs[j]:offs[j] + s // 16],
                    num_idxs=s,
                    num_idxs_reg=regs[s],
                    elem_size=C,
                    queue_num=j % 4,
                    single_packet=False,
                )
                if j == 3:
                    nc.scalar.dma_start(out=al_sb[:], in_=alpha_d.ap())
                    nc.scalar.dma_start(out=xt_sb[:], in_=xt_nm)
                    nc.scalar.add(out=al_sb[:], in_=al_sb[:], add=1.0)
                    for c in range(2):
                        nc.vector.tensor_scalar_mul(
                            out=o[c][:],
                            in0=xt_sb[:, c * segw:(c + 1) * segw],
                            scalar1=al_sb[:, :1],
                        )

            for j, s in enumerate(sizes):
                fb = 0  # free-block offset within g[j]
                for (k, lo, ln) in _bc_pieces(j):
                    # split the piece at the o-chunk boundary (i = 2048)
                    a = lo
                    while a < lo + ln:
                        c = a // 2048
                        b = min(lo + ln, (c + 1) * 2048)
                        nb = (b - a) // P
                        col0 = ((a - c * 2048) // P) * C
                        nc.vector.tensor_add(
                            out=o[c][:, col0:col0 + nb * C],
                            in0=o[c][:, col0:col0 + nb * C],
                            in1=g[j][:, fb * C:(fb + nb) * C],
                        )
                        fb += nb
                        a = b
            for c in range(2):
                nc.sync.dma_start(
                    out=out_nm[:, c * segw:(c + 1) * segw], in_=o[c][:],
                )

    nc.compile()
    _split_multiwaits(nc, mybir)
    return nc


def _build_program_hbm_ts():
    import concourse.mybir as mybir
    import concourse.tile as tile
    from concourse import bacc

    nc = bacc.Bacc("TRN2", target_bir_lowering=False, debug=False,
                   num_devices=NCORES, num_swdge_queues=4,
                   dynamic_dma_scratch_size=16384)
    total_cols = sum(ln // 16 for _, _, ln in _TS_BLOCKS)
    xt_d = nc.dram_tensor("xt", [N, C], mybir.dt.float32, kind="ExternalInput")
    idx_d = nc.dram_tensor("idx", [P, total_cols], mybir.dt.int16,
                           kind="ExternalInput")
    alpha_d = nc.dram_tensor("alpha", [P, 1], mybir.dt.float32,
                             kind="ExternalInput")
    out_d = nc.dram_tensor("out", [N, C], mybir.dt.float32,
                           kind="ExternalOutput")

    segw = (2048 // P) * C
    offs = []
    off = 0
    for _, _, ln in _TS_BLOCKS:
        offs.append(off)
        off += ln // 16

    with tile.TileContext(nc) as tc:
        with tc.tile_pool(name="sbuf", bufs=1) as pool:
            xt_sb = pool.tile([P, FREE], mybir.dt.float32, tag="xt")
            idx_sb = pool.tile([P, total_cols], mybir.dt.int16, tag="idx")
            al_sb = pool.tile([P, 1], mybir.dt.float32, tag="al")
            g = [pool.tile([P, (ln // P) * C], mybir.dt.float32,
                           tag=f"g{i}", name=f"g{i}")
                 for i, (_, _, ln) in enumerate(_TS_BLOCKS)]
            o = [pool.tile([P, segw], mybir.dt.float32, tag=f"o{c}",
                           name=f"o{c}") for c in range(2)]

            xt_nm = xt_d.ap().rearrange("(p a) c -> p (a c)", p=P)
            out_nm = out_d.ap().rearrange("(p a) c -> p (a c)", p=P)

            # fast idx for the first wave, rest of chunk-0 on scalar
            b4 = offs[4]
            b16 = offs[16]
            nc.sync.dma_start(out=idx_sb[:, :b4], in_=idx_d.ap()[:, :b4])
            nc.scalar.dma_start(out=idx_sb[:, b4:b16],
                                in_=idx_d.ap()[:, b4:b16])
            regs = {2048: nc.gpsimd.to_reg(2048), 1024: nc.gpsimd.to_reg(1024)}

            for i, (k, lo, ln) in enumerate(_TS_BLOCKS):
                nc.gpsimd.dma_gather(
                    out_ap=g[i][:].rearrange("p (a c) -> p a c", c=C),
                    in_ap=xt_d.ap(),
                    idxs_ap=idx_sb[:, offs[i]:offs[i] + ln // 16],
                    num_idxs=ln,
                    num_idxs_reg=regs[ln],
                    elem_size=C,
                    queue_num=i % 4,
                    single_packet=False,
                )
                if i == 15:
                    nc.sync.dma_start(out=idx_sb[:, b16:],
                                      in_=idx_d.ap()[:, b16:])
                    nc.scalar.dma_start(out=al_sb[:], in_=alpha_d.ap())
                    nc.scalar.dma_start(out=xt_sb[:], in_=xt_nm)
                    nc.scalar.add(out=al_sb[:], in_=al_sb[:], add=1.0)
                    for c in range(2):
                        nc.vector.tensor_scalar_mul(
                            out=o[c][:],
                            in0=xt_sb[:, c * segw:(c + 1) * segw],
                            scalar1=al_sb[:, :1],
                        )

            done = [0, 0]  # blocks consumed per chunk (for store ordering)
            for i, (k, lo, ln) in enumerate(_TS_BLOCKS):
                c = lo // 2048
                col0 = ((lo - c * 2048) // P) * C
                nc.vector.tensor_add(
                    out=o[c][:, col0:col0 + (ln // P) * C],
                    in0=o[c][:, col0:col0 + (ln // P) * C],
                    in1=g[i][:],
                )
            for c in range(2):
                nc.sync.dma_start(
                    out=out_nm[:, c * segw:(c + 1) * segw], in_=o[c][:],
                )

    nc.compile()
    _split_multiwaits(nc, mybir)
    return nc


# ---------------------------------------------------------------------------
# Device program — original HBM node-major f32 row-gather (fallback)
# ---------------------------------------------------------------------------
def _build_program_hbm():
    import concourse.mybir as mybir
    import concourse.tile as tile
    from concourse import bacc

    chunk = int(os.environ.get("KERNEL_HBM_CHUNK", "2048"))
    scratch = int(os.environ.get("KERNEL_HBM_SCRATCH", "16384"))
    gq = int(os.environ.get("KERNEL_GQ", "4"))
    priv = bool(int(os.environ.get("KERNEL_PRIV", "0")))
    ind = bool(int(os.environ.get("KERNEL_IND", "0")))
    nc = bacc.Bacc("TRN2", target_bir_lowering=False, debug=False,
                   num_devices=NCORES, num_swdge_queues=4,
                   dynamic_dma_scratch_size=scratch)
    cpk = N // chunk
    ipg = chunk // 16
    opg = chunk // P
    segw = opg * C

    xt_d = nc.dram_tensor("xt", [N, C], mybir.dt.float32, kind="ExternalInput")
    if ind:
        idx_d = nc.dram_tensor("idx", [P, K * cpk * opg], mybir.dt.int32,
                               kind="ExternalInput")
    else:
        idx_d = nc.dram_tensor("idx", [P, K * (N // 16)], mybir.dt.int16,
                               kind="ExternalInput")
    alpha_d = nc.dram_tensor("alpha", [P, 1], mybir.dt.float32,
                             kind="ExternalInput")
    out_d = nc.dram_tensor("out", [N, C], mybir.dt.float32,
                           kind="ExternalOutput")

    spare_ids = []
    SPARE_LO = None
    if priv:
        spares = [nc.alloc_semaphore(f"gpriv{i}")
                  for i in range(K * (N // chunk))]
        spare_ids = sorted(s.num for s in spares)
        SPARE_LO = spare_ids[0]

    with tile.TileContext(nc) as tc:
        with tc.tile_pool(name="sbuf", bufs=1) as pool:
            xt_sb = pool.tile([P, FREE], mybir.dt.float32, tag="xt")
            idt = mybir.dt.int32 if ind else mybir.dt.int16
            iw = opg if ind else ipg  # idx cols per (c, k) block
            idx_sb = [pool.tile([P, K * iw], idt, tag=f"idx{c}",
                                name=f"idx{c}") for c in range(cpk)]
            al_sb = pool.tile([P, 1], mybir.dt.float32, tag="al")
            g = [[pool.tile([P, segw], mybir.dt.float32, tag=f"g{k}_{c}",
                            name=f"g{k}_{c}") for c in range(cpk)]
                 for k in range(K)]
            o = [pool.tile([P, segw], mybir.dt.float32, tag=f"o{c}",
                           name=f"o{c}") for c in range(cpk)]

            xt_nm = xt_d.ap().rearrange("(p a) c -> p (a c)", p=P)
            out_nm = out_d.ap().rearrange("(p a) c -> p (a c)", p=P)

            # first gather wave (k=0..gq-1) gets its own small fast idx DMA;
            # the rest of chunk-0's idx arrives on the other HWDGE engine
            nc.sync.dma_start(
                out=idx_sb[0][:, :gq * iw],
                in_=idx_d.ap()[:, :gq * iw],
            )
            nc.scalar.dma_start(
                out=idx_sb[0][:, gq * iw:],
                in_=idx_d.ap()[:, gq * iw:K * iw],
            )
            nreg = nc.gpsimd.to_reg(chunk)

            gi = 0
            for c in range(cpk):
                for k in range(K):
                    if ind:
                        from concourse import bass as _bass
                        nc.gpsimd.indirect_dma_start(
                            out=g[k][c][:].rearrange(
                                "p (a c) -> p a c", c=C),
                            out_offset=None,
                            in_=xt_d.ap(),
                            in_offset=_bass.IndirectOffsetOnAxis(
                                ap=idx_sb[c][:, k * iw:(k + 1) * iw],
                                axis=0,
                            ),
                        )
                    else:
                        nc.gpsimd.dma_gather(
                            out_ap=g[k][c][:].rearrange(
                                "p (a c) -> p a c", c=C),
                            in_ap=xt_d.ap(),
                            idxs_ap=idx_sb[c][:, k * iw:(k + 1) * iw],
                            num_idxs=chunk,
                            num_idxs_reg=nreg,
                            elem_size=C,
                            queue_num=gi % gq,
                            single_packet=False,
                        )
                    gi += 1
                if c == 0:
                    for cc in range(1, cpk):
                        nc.sync.dma_start(
                            out=idx_sb[cc][:],
                            in_=idx_d.ap()[:, cc * K * iw:
                                           (cc + 1) * K * iw],
                        )
                    nc.scalar.dma_start(out=al_sb[:], in_=alpha_d.ap())
                    nc.scalar.dma_start(out=xt_sb[:], in_=xt_nm)
                    nc.scalar.add(out=al_sb[:], in_=al_sb[:], add=1.0)

            for c in range(cpk):
                nc.vector.tensor_scalar_mul(
                    out=o[c][:], in0=xt_sb[:, c * segw:(c + 1) * segw],
                    scalar1=al_sb[:, :1],
                )
                for k in range(K):
                    nc.vector.tensor_add(
                        out=o[c][:], in0=o[c][:], in1=g[k][c][:],
                    )
                nc.sync.dma_start(
                    out=out_nm[:, c * segw:(c + 1) * segw], in_=o[c][:],
                )

    nc.compile()
    if priv:
        # the spares must not collide with the tile framework's internal
        # lane sems (walrus-range ids below the alloc_semaphore range)
        used = set()
        for f in nc.m.functions:
            for bb in f.blocks:
                for ins in bb.instructions:
                    si = ins.sync_info
                    if si is None:
                        continue
                    used.update(w.id for w in (si.on_wait or [])
                                if w.sync_type == "semaphore")
                    used.update(u.id for u in (si.on_update or [])
                                if u.sync_type == "semaphore")
        bad = used.intersection(spare_ids)
        assert not bad, f"spare sems collide with program sems: {sorted(bad)}"
        nused = _privatize_gather_sems(nc, mybir, spare_ids)
        # include the spares in the teardown drain's sem reset range so they
        # are zeroed between NEFF executions (the sim's "cleared" invariant
        # and real multi-run correctness both come from this range)
        patched = False
        for f in nc.m.functions:
            for bb in f.blocks:
                for ins in bb.instructions:
                    if (type(ins).__name__ == "InstDrain"
                            and ins.is_reset_sema):
                        ins.reset_range_start = min(
                            ins.reset_range_start, spare_ids[0])
                        ins.reset_range_stop = max(
                            ins.reset_range_stop,
                            spare_ids[nused - 1] if nused else spare_ids[0])
                        patched = True
        assert patched, "no is_reset_sema drain found to widen"
    _split_multiwaits(nc, mybir)
    return nc


_PROGRAM = {}


def _get_program(mode):
    if mode not in _PROGRAM:
        if mode == "hbm":
            if bool(int(os.environ.get("KERNEL_BIGCHUNK", "0"))):
                _PROGRAM[mode] = _build_program_hbm_bc()
            elif bool(int(os.environ.get("KERNEL_TAILSPLIT", "1"))):
                _PROGRAM[mode] = _build_program_hbm_ts()
            else:
                _PROGRAM[mode] = _build_program_hbm()
        else:
            _PROGRAM[mode] = _build_program_t(src_sbuf=(mode == "sbuf"))
    return _PROGRAM[mode]


# ---------------------------------------------------------------------------
# Host glue
# ---------------------------------------------------------------------------
_slot = np.arange(N)
_PERM = (_slot % P) * COLS + (_slot // P)  # node id for flat gather slot i


def _prep_idx_ind(edge_b):
    """[N, K] int32 -> [128, cpk*K*opg] int32 for indirect_dma_start:
    block (c, k) col m on partition p = e[p*COLS + c*opg + m, k]."""
    chunk = int(os.environ.get("KERNEL_HBM_CHUNK", "2048"))
    cpk = N // chunk
    opg = chunk // P
    e = edge_b.astype(np.int32).reshape(P, cpk, opg, K)   # [p, c, m, k]
    w = np.transpose(e, (0, 1, 3, 2))                     # [p, c, k, m]
    return np.ascontiguousarray(w.reshape(P, cpk * K * opg))


def _prep_idx_hbm(edge_b):
    cpk = N // 2048
    ipg = 2048 // 16
    ids = edge_b[_PERM, :].astype(np.int16)
    f = ids.T.reshape(K, N // 16, 16)
    w = np.transpose(f, (2, 0, 1))
    w = np.tile(w, (8, 1, 1))
    w = w.reshape(P, K, cpk, ipg).transpose(0, 2, 1, 3)
    return np.ascontiguousarray(w.reshape(P, K * (N // 16)))


def _prep_idx_t(edge_b):
    """[N, K] int32 -> [128, NBLK*IPC] int16; block (c, k) holds the wrapped
    indices for gather chunk c of neighbor k: slot i (= node c*CHUNK+i) at
    partition i%16, col i//16, replicated x8 across partition groups."""
    e = edge_b.astype(np.int16)                       # [N, K]
    e = e.reshape(CPK, GATHER_CHUNK, K)               # [c, i, k]
    e = e.reshape(CPK, IPC, 16, K)                    # [c, col, p16, k]
    w = np.transpose(e, (2, 0, 3, 1))                 # [p16, c, k, col]
    w = np.tile(w, (8, 1, 1, 1))                      # [128, c, k, col]
    return np.ascontiguousarray(w.reshape(P, NBLK * IPC))


def _prep_tab(xb, sbuf_layout):
    """x[b] channel-major [C, N] f32 -> bf16 duplicated token table."""
    import ml_dtypes
    t = np.ascontiguousarray(xb.T).astype(ml_dtypes.bfloat16)  # [N, C]
    tok = np.concatenate([t, t], axis=1)                       # [N, 128]
    if not sbuf_layout:
        return tok
    # partition t%128, rank t//128
    return np.ascontiguousarray(
        tok.reshape(N // P, P, P).transpose(1, 0, 2).reshape(P, N))


def kernel(x, edge_index, alpha):
    global LAST_EXEC_NS
    _install_profile_shim()
    from concourse import bass_utils

    x = np.asarray(x)
    edge_index = np.asarray(edge_index)
    alpha_v = np.float32(np.asarray(alpha))
    mode = MODE

    nc = _get_program(mode)

    in_maps = []
    if mode == "hbm":
        ind = bool(int(os.environ.get("KERNEL_IND", "0")))
        ts = bool(int(os.environ.get("KERNEL_TAILSPLIT", "1")))
        bc = bool(int(os.environ.get("KERNEL_BIGCHUNK", "0")))
        prep = (_prep_idx_bc if bc
                else _prep_idx_blocks if ts
                else _prep_idx_ind if ind else _prep_idx_hbm)
        xt = np.transpose(x[..., 0], (0, 2, 1))  # [B, N, C]
        for b in range(B):
            in_maps.append({
                "xt": np.ascontiguousarray(xt[b]),
                "idx": prep(edge_index[b]),
                "alpha": np.full((P, 1), alpha_v, dtype=np.float32),
            })
    else:
        for b in range(B):
            xb = np.ascontiguousarray(x[b, :, :, 0])  # [C, N]
            in_maps.append({
                "tab": _prep_tab(xb, sbuf_layout=(mode == "sbuf")),
                "idx": _prep_idx_t(edge_index[b]),
                "x": xb,
                "alpha": np.full((P, 1), alpha_v, dtype=np.float32),
            })

    trace = bool(int(os.environ.get("KERNEL_PROFILE", "0")))
    res = bass_utils.run_bass_kernel_spmd(
        nc, in_maps, core_ids=list(range(NCORES)), trace=trace
    )
    LAST_EXEC_NS = res.exec_time_ns

    out = np.empty((B, C, N, 1), dtype=np.float32)
    for b in range(B):
        if mode == "hbm":
            out[b, :, :, 0] = res.results[b]["out"].T
        else:
            out[b, :, :, 0] = res.results[b]["out"]
    return out


# revision 41
# speedup vs baseline: 1.0413x; 1.0413x over previous
"""Trainium2 Bass kernel for nn_MessagePassingLayer (gnn_message_passing).

Computes, for x:[B,C,N,1] f32, edge_index:[B,N,K] i32, alpha scalar:
    out[b,c,n] = x[b,c,n]*(1+alpha) + sum_k x[b,c,edge_index[b,n,k]]

Sharding: B=8 batch samples, one per NeuronCore (data parallel). Edge
indices are intra-sample so there is no cross-core communication.

Modes (KERNEL_MODE):
  hbm (default): node-major f32 HBM row gather, 4 SWDGE queues, with a
    tail-split issue schedule (KERNEL_TAILSPLIT=1 default): the last
    wave uses 1024-idx gathers so the post-generation drain tail halves.
  sbuf/hbmt: experimental bf16 transpose-gather modes (correct only on
    a single queue due to shared-xbar pairing corruption; slow).
"""
import os
import sys
import types

import numpy as np

B, C, N, K = 8, 64, 4096, 16
NCORES = 8
P = 128
COLS = N // P  # 32 nodes per partition
FREE = COLS * C  # 2048 f32 per partition

LAST_EXEC_NS = None

MODE = os.environ.get("KERNEL_MODE", "hbm")


# ---------------------------------------------------------------------------
# axon NTFF profile hook shim (the agent image's antenv lacks axon_hooks)
# ---------------------------------------------------------------------------
def _install_profile_shim():
    if "antenv.axon_hooks" in sys.modules:
        return
    try:
        import antenv

        mod = types.ModuleType("antenv.axon_hooks")
        mod._hook = None
        mod.set_axon_ntff_profile_hook = lambda h: setattr(mod, "_hook", h)
        mod.get_axon_ntff_profile_hook = lambda: mod._hook
        sys.modules["antenv.axon_hooks"] = mod
        antenv.axon_hooks = mod
        from trn_agent_boot.trn_boot import _ntff_profile_via_ctypes

        mod.set_axon_ntff_profile_hook(
            _ntff_profile_via_ctypes("/opt/axon/libaxon_pjrt.so")
        )
    except Exception:
        pass


# ---------------------------------------------------------------------------
# Walrus in this container rejects >1 sync-wait per instruction. Split any
# multi-wait instruction into single-wait NoOps on the same engine.
# ---------------------------------------------------------------------------
def _split_multiwaits(nc, mybir):
    cnt = [0]
    for f in nc.m.functions:
        for bb in f.blocks:
            new_list = []
            for ins in bb.instructions:
                si = ins.sync_info
                if si is not None and si.on_wait and len(si.on_wait) > 1:
                    waits = list(si.on_wait)
                    for w in waits[:-1]:
                        cnt[0] += 1
                        nop = mybir.InstNoOp(name=f"I-waitsplit-{cnt[0]}")
                        nop.engine = ins.engine
                        nop.sync_info = mybir.SyncInfo(on_wait=[w], on_update=[])
                        try:
                            nc.register_instruction(nop, overwrite=True)
                        except Exception:
                            pass
                        new_list.append(nop)
                    ins.sync_info = mybir.SyncInfo(
                        on_wait=[waits[-1]], on_update=list(si.on_update)
                    )
                new_list.append(ins)
            bb.instructions = new_list


# ---------------------------------------------------------------------------
# Device program — sbuf/hbmt transpose-gather modes
# ---------------------------------------------------------------------------
GATHER_CHUNK = int(os.environ.get("KERNEL_GATHER_CHUNK", "2048"))
SCRATCH = int(os.environ.get("KERNEL_SCRATCH", "32768"))
NQUEUES = int(os.environ.get("KERNEL_QUEUES", "4"))

CPK = N // GATHER_CHUNK          # chunks per k
IPC = GATHER_CHUNK // 16         # idx cols per chunk (16-partition wrap)
NBLK = K * CPK                   # total gather instructions


def _build_program_t(src_sbuf: bool):
    import concourse.mybir as mybir
    import concourse.tile as tile
    from concourse import bacc

    nc = bacc.Bacc("TRN2", target_bir_lowering=False, debug=False,
                   num_devices=NCORES, num_swdge_queues=NQUEUES,
                   dynamic_dma_scratch_size=SCRATCH)
    bf16 = mybir.dt.bfloat16
    f32 = mybir.dt.float32
    # token table: node t -> 256B = 128 bf16 (channels 0..63 duplicated).
    # SBUF layout: partition t%128, free cols [ (t//128)*128, +128 ).
    # HBM layout (hbmt): row-major [4096, 128].
    if src_sbuf:
        tab_d = nc.dram_tensor("tab", [P, N // P * P], bf16,
                               kind="ExternalInput")
    else:
        tab_d = nc.dram_tensor("tab", [N, P], bf16, kind="ExternalInput")
    idx_d = nc.dram_tensor("idx", [P, NBLK * IPC], mybir.dt.int16,
                           kind="ExternalInput")
    x_d = nc.dram_tensor("x", [C, N], f32, kind="ExternalInput")
    al_d = nc.dram_tensor("alpha", [P, 1], f32, kind="ExternalInput")
    out_d = nc.dram_tensor("out", [C, N], f32, kind="ExternalOutput")

    with tile.TileContext(nc) as tc:
        with tc.tile_pool(name="sbuf", bufs=1) as pool:
            idx_sb = pool.tile([P, NBLK * IPC], mybir.dt.int16, tag="idx")
            x_sb = pool.tile([C, N], f32, tag="x")
            al_sb = pool.tile([P, 1], f32, tag="al")
            g = [[pool.tile([P, GATHER_CHUNK], bf16, tag=f"g{k}_{c}",
                            name=f"g{k}_{c}") for c in range(CPK)]
                 for k in range(K)]
            if src_sbuf:
                tab_sb = pool.tile([P, N], bf16, tag="tab")
                nc.sync.dma_start(out=tab_sb[:], in_=tab_d.ap())

            # idx chunk-0 blocks first so the first gather wave starts ASAP
            nc.scalar.dma_start(out=idx_sb[:, :K * IPC],
                                in_=idx_d.ap()[:, :K * IPC])
            nreg = nc.gpsimd.to_reg(GATHER_CHUNK)

            gi = 0
            for c in range(CPK):
                for k in range(K):
                    blk = c * K + k
                    src = tab_sb[:] if src_sbuf else tab_d.ap()
                    kw = dict(sbuf_tokens_per_rank=P,
                              sbuf_free_dim_per_rank=256) if src_sbuf else {}
                    nc.gpsimd.dma_gather(
                        out_ap=g[k][c][:].rearrange("p (a n) -> p a n", a=1),
                        in_ap=src,
                        idxs_ap=idx_sb[:, blk * IPC:(blk + 1) * IPC],
                        num_idxs=GATHER_CHUNK,
                        num_idxs_reg=nreg,
                        elem_size=P,
                        transpose=True,
                        queue_num=gi % NQUEUES,
                        single_packet=False,
                        **kw,
                    )
                    gi += 1
                if c == 0:
                    # remaining loads, issued behind the first gather wave
                    nc.scalar.dma_start(out=al_sb[:], in_=al_d.ap())
                    if CPK > 1:
                        nc.scalar.dma_start(
                            out=idx_sb[:, K * IPC:],
                            in_=idx_d.ap()[:, K * IPC:])
                    nc.scalar.dma_start(out=x_sb[:], in_=x_d.ap())
                    nc.scalar.add(out=al_sb[:], in_=al_sb[:], add=1.0)
                    # x*(1+alpha) on DVE while gathers run
                    nc.vector.tensor_scalar_mul(
                        out=x_sb[:], in0=x_sb[:], scalar1=al_sb[:C, :1])

            # pairwise tree sum per chunk (bf16), then out = xs + m
            for c in range(CPK):
                for step in (1, 2, 4, 8):
                    for a in range(0, K, 2 * step):
                        nc.vector.tensor_add(
                            out=g[a][c][:], in0=g[a][c][:],
                            in1=g[a + step][c][:])
                lo, hi = c * GATHER_CHUNK, (c + 1) * GATHER_CHUNK
                nc.vector.tensor_add(
                    out=x_sb[:, lo:hi], in0=x_sb[:, lo:hi],
                    in1=g[0][c][:C, :])
                nc.sync.dma_start(out=out_d.ap()[:, lo:hi],
                                  in_=x_sb[:, lo:hi])

    nc.compile()
    _split_multiwaits(nc, mybir)
    return nc


# ---------------------------------------------------------------------------
# Post-compile pass: give every SWDGE gather a private completion semaphore.
#
# The tile framework round-robins all Pool DMA completions onto 8 DMASW lane
# sems; lane reuse puts a wait on each gather past the 8th, which head-blocks
# the GpSimd sequencer and caps DMA in-flight depth (~71% SDMA duty in the
# trace). With one sem per gather there is no reuse wait: issue depth is
# bounded only by the descriptor rings, and SDMA stays fed.
# ---------------------------------------------------------------------------
def _privatize_gather_sems(nc, mybir, spare_ids):
    import dataclasses

    count = {}   # lane sem id -> max generation seen
    remap = {}   # (lane, gen) -> private sem id
    nxt = 0
    for f in nc.m.functions:
        for bb in f.blocks:
            for ins in bb.instructions:
                if type(ins).__name__ not in ("InstDMAGatherAnt",
                                              "InstDMACopy"):
                    continue
                if str(getattr(ins, "engine", "")) != "EngineType.Pool":
                    continue
                si = ins.sync_info
                upd = [u for u in si.on_update
                       if u.sync_type == "semaphore"
                       and (u.ant_name or "").startswith("DMASW")]
                if not upd:
                    continue
                u = upd[0]
                lane = u.id
                # generation = (value of the reuse wait on its own lane)/16+1;
                # gen 1 has no reuse wait. Iteration order is NOT schedule
                # order, so the wait value is the only reliable source.
                reuse = [w for w in si.on_wait
                         if w.sync_type == "semaphore" and w.id == lane]
                gen = (reuse[0].wait_value // 16 + 1) if reuse else 1
                count[lane] = max(count.get(lane, 0), gen)
                if gen == 1:
                    continue
                new_id = spare_ids[nxt]
                nxt += 1
                remap[(lane, gen)] = new_id
                new_upds = [
                    dataclasses.replace(x, id=new_id, ant_name=f"GPRIV{nxt}")
                    if x is u else x
                    for x in si.on_update
                ]
                new_waits = [w for w in si.on_wait
                             if not (w.sync_type == "semaphore"
                                     and w.id == lane)]
                ins.sync_info = mybir.SyncInfo(on_wait=new_waits,
                                               on_update=new_upds)
    # consumers + teardown: (lane, 16k) -> (lane, 16) + privates of gens 2..k
    for f in nc.m.functions:
        for bb in f.blocks:
            for ins in bb.instructions:
                si = ins.sync_info
                if si is None or not si.on_wait:
                    continue
                if type(ins).__name__ == "InstDMAGatherAnt":
                    continue
                waits, changed = [], False
                for w in si.on_wait:
                    if (w.sync_type == "semaphore" and w.id in count
                            and w.wait_mode == "sem-ge-imm"
                            and w.wait_value % 16 == 0):
                        k = w.wait_value // 16
                        if k >= 2:
                            changed = True
                            waits.append(dataclasses.replace(w, wait_value=16))
                            for j in range(2, k + 1):
                                waits.append(dataclasses.replace(
                                    w, id=remap[(w.id, j)],
                                    ant_name=f"GPRIVW{w.id}_{j}",
                                    wait_value=16))
                            continue
                    waits.append(w)
                if changed:
                    ins.sync_info = mybir.SyncInfo(
                        on_wait=waits, on_update=list(si.on_update))
    return nxt


# ---------------------------------------------------------------------------
# Tail-split variant: identical to hbm mode, but the final issue wave uses
# 1024-idx gathers so the post-generation drain tail halves (~5 µs).
# Blocks are (k, lo_slot, n_slots) in issue order; lo must be 0 mod 128.
# ---------------------------------------------------------------------------
_TS_BLOCKS = (
    [(k, 0, 2048) for k in range(16)]
    + [(k, 2048, 2048) for k in range(12)]
    + [(k, 2048, 1024) for k in range(12, 16)]
    + [(k, 3072, 1024) for k in range(12, 16)]
)


def _prep_idx_blocks(edge_b):
    parts = []
    for (k, lo, ln) in _TS_BLOCKS:
        ids = edge_b[_PERM[lo:lo + ln], k].astype(np.int16)   # [ln]
        w = ids.reshape(ln // 16, 16).T                       # [16, ln/16]
        parts.append(np.tile(w, (8, 1)))                      # [128, ln/16]
    return np.ascontiguousarray(np.concatenate(parts, axis=1))


# Big-chunk variant: flat global slot space G = k*4096 + slot, cut into
# 3968-slot instructions (ring fits 3968/16+1=249 <= 256 descs/engine;
# only 4096 -> 257 overflows). 17 instructions instead of 32.
_BC_CHUNK = 3968
# 16x3968 + 4x512: 20 instructions, exactly 5 per queue (balanced gen),
# tiny 4x512 final wave (minimal post-generation drain tail)
_BC_BOUNDS = (list(range(0, 16 * _BC_CHUNK + 1, _BC_CHUNK))
              + [16 * _BC_CHUNK + 512 * i for i in range(1, 5)])


def _bc_pieces(j):
    """Instruction j's (k, i_lo, i_len) pieces in the per-k slot space."""
    glo, ghi = _BC_BOUNDS[j], _BC_BOUNDS[j + 1]
    out = []
    g = glo
    while g < ghi:
        k, i = g // N, g % N
        ln = min(ghi - g, N - i)
        out.append((k, i, ln))
        g += ln
    return out


def _prep_idx_bc(edge_b):
    parts = []
    for j in range(len(_BC_BOUNDS) - 1):
        vals = []
        for (k, lo, ln) in _bc_pieces(j):
            vals.append(edge_b[_PERM[lo:lo + ln], k].astype(np.int16))
        v = np.concatenate(vals)                      # [inst slots]
        w = v.reshape(len(v) // 16, 16).T             # [16, slots/16]
        parts.append(np.tile(w, (8, 1)))
    return np.ascontiguousarray(np.concatenate(parts, axis=1))


def _build_program_hbm_bc():
    import concourse.mybir as mybir
    import concourse.tile as tile
    from concourse import bacc

    nc = bacc.Bacc("TRN2", target_bir_lowering=False, debug=False,
                   num_devices=NCORES, num_swdge_queues=4,
                   dynamic_dma_scratch_size=16384)
    nins = len(_BC_BOUNDS) - 1
    sizes = [_BC_BOUNDS[j + 1] - _BC_BOUNDS[j] for j in range(nins)]
    total_cols = sum(s // 16 for s in sizes)
    xt_d = nc.dram_tensor("xt", [N, C], mybir.dt.float32, kind="ExternalInput")
    idx_d = nc.dram_tensor("idx", [P, total_cols], mybir.dt.int16,
                           kind="ExternalInput")
    alpha_d = nc.dram_tensor("alpha", [P, 1], mybir.dt.float32,
                             kind="ExternalInput")
    out_d = nc.dram_tensor("out", [N, C], mybir.dt.float32,
                           kind="ExternalOutput")

    segw = (2048 // P) * C
    offs = []
    off = 0
    for s in sizes:
        offs.append(off)
        off += s // 16

    with tile.TileContext(nc) as tc:
        with tc.tile_pool(name="sbuf", bufs=1) as pool:
            xt_sb = pool.tile([P, FREE], mybir.dt.float32, tag="xt")
            idx_sb = pool.tile([P, total_cols], mybir.dt.int16, tag="idx")
            al_sb = pool.tile([P, 1], mybir.dt.float32, tag="al")
            g = [pool.tile([P, (s // P) * C], mybir.dt.float32,
                           tag=f"g{j}", name=f"g{j}")
                 for j, s in enumerate(sizes)]
            o = [pool.tile([P, segw], mybir.dt.float32, tag=f"o{c}",
                           name=f"o{c}") for c in range(2)]

            xt_nm = xt_d.ap().rearrange("(p a) c -> p (a c)", p=P)
            out_nm = out_d.ap().rearrange("(p a) c -> p (a c)", p=P)

            b4 = offs[4] if nins > 4 else total_cols
            nc.sync.dma_start(out=idx_sb[:, :b4], in_=idx_d.ap()[:, :b4])
            if b4 < total_cols:
                nc.scalar.dma_start(out=idx_sb[:, b4:],
                                    in_=idx_d.ap()[:, b4:])
            regs = {s: nc.gpsimd.to_reg(s) for s in sorted(set(sizes))}

            for j, s in enumerate(sizes):
                nc.gpsimd.dma_gather(
                    out_ap=g[j][:].rearrange("p (a c) -> p a c", c=C),
                    in_ap=xt_d.ap(),
                    idxs_ap=idx_sb[:, offs[j]:offs[j] + s // 16],
                    num_idxs=s,
                    num_idxs_reg=regs[s],
                    elem_size=C,
                    queue_num=j % 4,
                    single_packet=False,
                )
                if j == 3:
                    nc.scalar.dma_start(out=al_sb[:], in_=alpha_d.ap())
                    nc.scalar.dma_start(out=xt_sb[:], in_=xt_nm)
                    nc.scalar.add(out=al_sb[:], in_=al_sb[:], add=1.0)
                    for c in range(2):
                        nc.vector.tensor_scalar_mul(
                            out=o[c][:],
                            in0=xt_sb[:, c * segw:(c + 1) * segw],
                            scalar1=al_sb[:, :1],
                        )

            for j, s in enumerate(sizes):
                fb = 0  # free-block offset within g[j]
                for (k, lo, ln) in _bc_pieces(j):
                    # split the piece at the o-chunk boundary (i = 2048)
                    a = lo
                    while a < lo + ln:
                        c = a // 2048
                        b = min(lo + ln, (c + 1) * 2048)
                        nb = (b - a) // P
                        col0 = ((a - c * 2048) // P) * C
                        nc.vector.tensor_add(
                            out=o[c][:, col0:col0 + nb * C],
                            in0=o[c][:, col0:col0 + nb * C],
                            in1=g[j][:, fb * C:(fb + nb) * C],
                        )
                        fb += nb
                        a = b
            for c in range(2):
                nc.sync.dma_start(
                    out=out_nm[:, c * segw:(c + 1) * segw], in_=o[c][:],
                )

    nc.compile()
    _split_multiwaits(nc, mybir)
    return nc


def _build_program_hbm_ts():
    import concourse.mybir as mybir
    import concourse.tile as tile
    from concourse import bacc

    nc = bacc.Bacc("TRN2", target_bir_lowering=False, debug=False,
                   num_devices=NCORES, num_swdge_queues=4,
                   dynamic_dma_scratch_size=16384)
    total_cols = sum(ln // 16 for _, _, ln in _TS_BLOCKS)
    xt_d = nc.dram_tensor("xt", [N, C], mybir.dt.float32, kind="ExternalInput")
    idx_d = nc.dram_tensor("idx", [P, total_cols], mybir.dt.int16,
                           kind="ExternalInput")
    alpha_d = nc.dram_tensor("alpha", [P, 1], mybir.dt.float32,
                             kind="ExternalInput")
    out_d = nc.dram_tensor("out", [N, C], mybir.dt.float32,
                           kind="ExternalOutput")

    segw = (2048 // P) * C
    offs = []
    off = 0
    for _, _, ln in _TS_BLOCKS:
        offs.append(off)
        off += ln // 16

    with tile.TileContext(nc) as tc:
        with tc.tile_pool(name="sbuf", bufs=1) as pool:
            xt_sb = pool.tile([P, FREE], mybir.dt.float32, tag="xt")
            idx_sb = pool.tile([P, total_cols], mybir.dt.int16, tag="idx")
            al_sb = pool.tile([P, 1], mybir.dt.float32, tag="al")
            g = [pool.tile([P, (ln // P) * C], mybir.dt.float32,
                           tag=f"g{i}", name=f"g{i}")
                 for i, (_, _, ln) in enumerate(_TS_BLOCKS)]
            o = [pool.tile([P, segw], mybir.dt.float32, tag=f"o{c}",
                           name=f"o{c}") for c in range(2)]

            xt_nm = xt_d.ap().rearrange("(p a) c -> p (a c)", p=P)
            out_nm = out_d.ap().rearrange("(p a) c -> p (a c)", p=P)

            # fast idx for the first wave, rest of chunk-0 on scalar
            b4 = offs[4]
            b16 = offs[16]
            nc.sync.dma_start(out=idx_sb[:, :b4], in_=idx_d.ap()[:, :b4])
            nc.scalar.dma_start(out=idx_sb[:, b4:b16],
                                in_=idx_d.ap()[:, b4:b16])
            regs = {2048: nc.gpsimd.to_reg(2048), 1024: nc.gpsimd.to_reg(1024)}

            for i, (k, lo, ln) in enumerate(_TS_BLOCKS):
                nc.gpsimd.dma_gather(
                    out_ap=g[i][:].rearrange("p (a c) -> p a c", c=C),
                    in_ap=xt_d.ap(),
                    idxs_ap=idx_sb[:, offs[i]:offs[i] + ln // 16],
                    num_idxs=ln,
                    num_idxs_reg=regs[ln],
                    elem_size=C,
                    queue_num=i % 4,
                    single_packet=False,
                )
                if i == 15:
                    nc.sync.dma_start(out=idx_sb[:, b16:],
                                      in_=idx_d.ap()[:, b16:])
                    nc.scalar.dma_start(out=al_sb[:], in_=alpha_d.ap())
                    nc.scalar.dma_start(out=xt_sb[:], in_=xt_nm)
                    nc.scalar.add(out=al_sb[:], in_=al_sb[:], add=1.0)
                    for c in range(2):
                        nc.vector.tensor_scalar_mul(
                            out=o[c][:],
                            in0=xt_sb[:, c * segw:(c + 1) * segw],
                            scalar1=al_sb[:, :1],
                        )

            done = [0, 0]  # blocks consumed per chunk (for store ordering)
            for i, (k, lo, ln) in enumerate(_TS_BLOCKS):
                c = lo // 2048
                col0 = ((lo - c * 2048) // P) * C
                nc.vector.tensor_add(
                    out=o[c][:, col0:col0 + (ln // P) * C],
                    in0=o[c][:, col0:col0 + (ln // P) * C],
                    in1=g[i][:],
                )
            for c in range(2):
                nc.sync.dma_start(
                    out=out_nm[:, c * segw:(c + 1) * segw], in_=o[c][:],
                )

    nc.compile()
    _split_multiwaits(nc, mybir)
    return nc


# ---------------------------------------------------------------------------
# Device program — original HBM node-major f32 row-gather (fallback)
# ---------------------------------------------------------------------------
def _build_program_hbm():
    import concourse.mybir as mybir
    import concourse.tile as tile
    from concourse import bacc

    chunk = int(os.environ.get("KERNEL_HBM_CHUNK", "2048"))
    scratch = int(os.environ.get("KERNEL_HBM_SCRATCH", "16384"))
    gq = int(os.environ.get("KERNEL_GQ", "4"))
    priv = bool(int(os.environ.get("KERNEL_PRIV", "0")))
    ind = bool(int(os.environ.get("KERNEL_IND", "0")))
    nc = bacc.Bacc("TRN2", target_bir_lowering=False, debug=False,
                   num_devices=NCORES, num_swdge_queues=4,
                   dynamic_dma_scratch_size=scratch)
    cpk = N // chunk
    ipg = chunk // 16
    opg = chunk // P
    segw = opg * C

    xt_d = nc.dram_tensor("xt", [N, C], mybir.dt.float32, kind="ExternalInput")
    if ind:
        idx_d = nc.dram_tensor("idx", [P, K * cpk * opg], mybir.dt.int32,
                               kind="ExternalInput")
    else:
        idx_d = nc.dram_tensor("idx", [P, K * (N // 16)], mybir.dt.int16,
                               kind="ExternalInput")
    alpha_d = nc.dram_tensor("alpha", [P, 1], mybir.dt.float32,
                             kind="ExternalInput")
    out_d = nc.dram_tensor("out", [N, C], mybir.dt.float32,
                           kind="ExternalOutput")

    spare_ids = []
    SPARE_LO = None
    if priv:
        spares = [nc.alloc_semaphore(f"gpriv{i}")
                  for i in range(K * (N // chunk))]
        spare_ids = sorted(s.num for s in spares)
        SPARE_LO = spare_ids[0]

    with tile.TileContext(nc) as tc:
        with tc.tile_pool(name="sbuf", bufs=1) as pool:
            xt_sb = pool.tile([P, FREE], mybir.dt.float32, tag="xt")
            idt = mybir.dt.int32 if ind else mybir.dt.int16
            iw = opg if ind else ipg  # idx cols per (c, k) block
            idx_sb = [pool.tile([P, K * iw], idt, tag=f"idx{c}",
                                name=f"idx{c}") for c in range(cpk)]
            al_sb = pool.tile([P, 1], mybir.dt.float32, tag="al")
            g = [[pool.tile([P, segw], mybir.dt.float32, tag=f"g{k}_{c}",
                            name=f"g{k}_{c}") for c in range(cpk)]
                 for k in range(K)]
            o = [pool.tile([P, segw], mybir.dt.float32, tag=f"o{c}",
                           name=f"o{c}") for c in range(cpk)]

            xt_nm = xt_d.ap().rearrange("(p a) c -> p (a c)", p=P)
            out_nm = out_d.ap().rearrange("(p a) c -> p (a c)", p=P)

            # first gather wave (k=0..gq-1) gets its own small fast idx DMA;
            # the rest of chunk-0's idx arrives on the other HWDGE engine
            nc.sync.dma_start(
                out=idx_sb[0][:, :gq * iw],
                in_=idx_d.ap()[:, :gq * iw],
            )
            nc.scalar.dma_start(
                out=idx_sb[0][:, gq * iw:],
                in_=idx_d.ap()[:, gq * iw:K * iw],
            )
            nreg = nc.gpsimd.to_reg(chunk)

            gi = 0
            for c in range(cpk):
                for k in range(K):
                    if ind:
                        from concourse import bass as _bass
                        nc.gpsimd.indirect_dma_start(
                            out=g[k][c][:].rearrange(
                                "p (a c) -> p a c", c=C),
                            out_offset=None,
                            in_=xt_d.ap(),
                            in_offset=_bass.IndirectOffsetOnAxis(
                                ap=idx_sb[c][:, k * iw:(k + 1) * iw],
                                axis=0,
                            ),
                        )
                    else:
                        nc.gpsimd.dma_gather(
                            out_ap=g[k][c][:].rearrange(
                                "p (a c) -> p a c", c=C),
                            in_ap=xt_d.ap(),
                            idxs_ap=idx_sb[c][:, k * iw:(k + 1) * iw],
                            num_idxs=chunk,
                            num_idxs_reg=nreg,
                            elem_size=C,
                            queue_num=gi % gq,
                            single_packet=False,
                        )
                    gi += 1
                if c == 0:
                    for cc in range(1, cpk):
                        nc.sync.dma_start(
                            out=idx_sb[cc][:],
                            in_=idx_d.ap()[:, cc * K * iw:
                                           (cc + 1) * K * iw],
                        )
                    nc.scalar.dma_start(out=al_sb[:], in_=alpha_d.ap())
                    nc.scalar.dma_start(out=xt_sb[:], in_=xt_nm)
                    nc.scalar.add(out=al_sb[:], in_=al_sb[:], add=1.0)

            for c in range(cpk):
                nc.vector.tensor_scalar_mul(
                    out=o[c][:], in0=xt_sb[:, c * segw:(c + 1) * segw],
                    scalar1=al_sb[:, :1],
                )
                for k in range(K):
                    nc.vector.tensor_add(
                        out=o[c][:], in0=o[c][:], in1=g[k][c][:],
                    )
                nc.sync.dma_start(
                    out=out_nm[:, c * segw:(c + 1) * segw], in_=o[c][:],
                )

    nc.compile()
    if priv:
        # the spares must not collide with the tile framework's internal
        # lane sems (walrus-range ids below the alloc_semaphore range)
        used = set()
        for f in nc.m.functions:
            for bb in f.blocks:
                for ins in bb.instructions:
                    si = ins.sync_info
                    if si is None:
                        continue
                    used.update(w.id for w in (si.on_wait or [])
                                if w.sync_type == "semaphore")
                    used.update(u.id for u in (si.on_update or [])
                                if u.sync_type == "semaphore")
        bad = used.intersection(spare_ids)
        assert not bad, f"spare sems collide with program sems: {sorted(bad)}"
        nused = _privatize_gather_sems(nc, mybir, spare_ids)
        # include the spares in the teardown drain's sem reset range so they
        # are zeroed between NEFF executions (the sim's "cleared" invariant
        # and real multi-run correctness both come from this range)
        patched = False
        for f in nc.m.functions:
            for bb in f.blocks:
                for ins in bb.instructions:
                    if (type(ins).__name__ == "InstDrain"
                            and ins.is_reset_sema):
                        ins.reset_range_start = min(
                            ins.reset_range_start, spare_ids[0])
                        ins.reset_range_stop = max(
                            ins.reset_range_stop,
                            spare_ids[nused - 1] if nused else spare_ids[0])
                        patched = True
        assert patched, "no is_reset_sema drain found to widen"
    _split_multiwaits(nc, mybir)
    return nc


_PROGRAM = {}


def _get_program(mode):
    if mode not in _PROGRAM:
        if mode == "hbm":
            if bool(int(os.environ.get("KERNEL_BIGCHUNK", "0"))):
                _PROGRAM[mode] = _build_program_hbm_bc()
            elif bool(int(os.environ.get("KERNEL_TAILSPLIT", "1"))):
                _PROGRAM[mode] = _build_program_hbm_ts()
            else:
                _PROGRAM[mode] = _build_program_hbm()
        else:
            _PROGRAM[mode] = _build_program_t(src_sbuf=(mode == "sbuf"))
    return _PROGRAM[mode]


# ---------------------------------------------------------------------------
# Host glue
# ---------------------------------------------------------------------------
_slot = np.arange(N)
_PERM = (_slot % P) * COLS + (_slot // P)  # node id for flat gather slot i


def _prep_idx_ind(edge_b):
    """[N, K] int32 -> [128, cpk*K*opg] int32 for indirect_dma_start:
    block (c, k) col m on partition p = e[p*COLS + c*opg + m, k]."""
    chunk = int(os.environ.get("KERNEL_HBM_CHUNK", "2048"))
    cpk = N // chunk
    opg = chunk // P
    e = edge_b.astype(np.int32).reshape(P, cpk, opg, K)   # [p, c, m, k]
    w = np.transpose(e, (0, 1, 3, 2))                     # [p, c, k, m]
    return np.ascontiguousarray(w.reshape(P, cpk * K * opg))


def _prep_idx_hbm(edge_b):
    cpk = N // 2048
    ipg = 2048 // 16
    ids = edge_b[_PERM, :].astype(np.int16)
    f = ids.T.reshape(K, N // 16, 16)
    w = np.transpose(f, (2, 0, 1))
    w = np.tile(w, (8, 1, 1))
    w = w.reshape(P, K, cpk, ipg).transpose(0, 2, 1, 3)
    return np.ascontiguousarray(w.reshape(P, K * (N // 16)))


def _prep_idx_t(edge_b):
    """[N, K] int32 -> [128, NBLK*IPC] int16; block (c, k) holds the wrapped
    indices for gather chunk c of neighbor k: slot i (= node c*CHUNK+i) at
    partition i%16, col i//16, replicated x8 across partition groups."""
    e = edge_b.astype(np.int16)                       # [N, K]
    e = e.reshape(CPK, GATHER_CHUNK, K)               # [c, i, k]
    e = e.reshape(CPK, IPC, 16, K)                    # [c, col, p16, k]
    w = np.transpose(e, (2, 0, 3, 1))                 # [p16, c, k, col]
    w = np.tile(w, (8, 1, 1, 1))                      # [128, c, k, col]
    return np.ascontiguousarray(w.reshape(P, NBLK * IPC))


def _prep_tab(xb, sbuf_layout):
    """x[b] channel-major [C, N] f32 -> bf16 duplicated token table."""
    import ml_dtypes
    t = np.ascontiguousarray(xb.T).astype(ml_dtypes.bfloat16)  # [N, C]
    tok = np.concatenate([t, t], axis=1)                       # [N, 128]
    if not sbuf_layout:
        return tok
    # partition t%128, rank t//128
    return np.ascontiguousarray(
        tok.reshape(N // P, P, P).transpose(1, 0, 2).reshape(P, N))


def kernel(x, edge_index, alpha):
    global LAST_EXEC_NS
    _install_profile_shim()
    from concourse import bass_utils

    x = np.asarray(x)
    edge_index = np.asarray(edge_index)
    alpha_v = np.float32(np.asarray(alpha))
    mode = MODE

    nc = _get_program(mode)

    in_maps = []
    if mode == "hbm":
        ind = bool(int(os.environ.get("KERNEL_IND", "0")))
        ts = bool(int(os.environ.get("KERNEL_TAILSPLIT", "1")))
        bc = bool(int(os.environ.get("KERNEL_BIGCHUNK", "0")))
        prep = (_prep_idx_bc if bc
                else _prep_idx_blocks if ts
                else _prep_idx_ind if ind else _prep_idx_hbm)
        xt = np.transpose(x[..., 0], (0, 2, 1))  # [B, N, C]
        for b in range(B):
            in_maps.append({
                "xt": np.ascontiguousarray(xt[b]),
                "idx": prep(edge_index[b]),
                "alpha": np.full((P, 1), alpha_v, dtype=np.float32),
            })
    else:
        for b in range(B):
            xb = np.ascontiguousarray(x[b, :, :, 0])  # [C, N]
            in_maps.append({
                "tab": _prep_tab(xb, sbuf_layout=(mode == "sbuf")),
                "idx": _prep_idx_t(edge_index[b]),
                "x": xb,
                "alpha": np.full((P, 1), alpha_v, dtype=np.float32),
            })

    trace = bool(int(os.environ.get("KERNEL_PROFILE", "0")))
    res = bass_utils.run_bass_kernel_spmd(
        nc, in_maps, core_ids=list(range(NCORES)), trace=trace
    )
    LAST_EXEC_NS = res.exec_time_ns

    out = np.empty((B, C, N, 1), dtype=np.float32)
    for b in range(B):
        if mode == "hbm":
            out[b, :, :, 0] = res.results[b]["out"].T
        else:
            out[b, :, :, 0] = res.results[b]["out"]
    return out
